# revision 2
# baseline (speedup 1.0000x reference)
"""Distributed Trainium2 kernel for nn_ALEError_23742579212666.

Computes: loss = 0.7 * masked_mean((target-pred)^2, target>0)
               + 0.3 * mean(sobel(target) - sobel(pred))

Math notes:
  * sobel is linear with symmetric padding, so
    mean(sobel(t) - sobel(p)) = mean(sobel(t-p)) and the column-sum of the
    separable stencil collapses: smoothing [1,2,1] contributes a factor 4
    per axis (B,C,H -> 4^3 = 64), the derivative [-1,0,1] along W has
    column weights [-2, 0, ..., 0, +2].  Hence
      mean(sobel(d)) = 128 * sum_rows(d[..., W-1] - d[..., 0]) / N.
  * masked sum of squares expands as
      sum m*(t-p)^2 = sum relu(t)^2 - 2*sum relu(t)*p + sum m*p^2
    with m = (t > 0), using t*m == relu(t) exactly.  The two cross terms
    are computed on the TensorEngine as accumulated 128x128 "diagonal"
    matmuls (trace of X^T Y); relu(t)^2 uses the ScalarEngine's fused
    per-partition accumulator; the mask count comes from the VectorEngine's
    tensor_scalar accumulator.

Sharding: pure data parallel over batch, 4 images per core; per-core
partial sums (a few floats) are combined on the host (an on-device
all-reduce has a ~20us latency floor, ~half the whole kernel runtime).
"""

import sys

import numpy as np

if "/opt/trn_rl_repo" not in sys.path:
    sys.path.insert(0, "/opt/trn_rl_repo")

B, C, H, W = 32, 1, 512, 1024
NCORES = 8
BP = B // NCORES                 # batches per core
FOLD = 2                         # W-rows folded per SBUF row
RT = BP * C * H // FOLD          # 1024 DRAM rows per core (folded view)
TW = W * FOLD                    # 2048
P = 128                          # SBUF partitions
NT = RT // P                     # 8 tiles per tensor per core
NCH = TW // P                    # 16 column chunks per tile for diag matmuls
NTOT = float(B * C * H * W)      # 16777216
ALPHA = 0.3

_CACHE = {}


def _build_nc():
    from concourse import bacc, mybir, tile

    f32 = mybir.dt.float32
    bf16 = mybir.dt.bfloat16
    Act = mybir.ActivationFunctionType
    Alu = mybir.AluOpType
    AxX = mybir.AxisListType.X

    nc = bacc.Bacc("TRN2", target_bir_lowering=False, debug=False,
                   num_devices=NCORES)
    t_ext = nc.declare_dram_parameter("target", [RT, TW], f32, isOutput=False)
    p_ext = nc.declare_dram_parameter("pred", [RT, TW], f32, isOutput=False)
    id_ext = nc.declare_dram_parameter("ident", [P, P], bf16, isOutput=False)
    out_ext = nc.declare_dram_parameter("out", [1, 16], f32, isOutput=True)

    with tile.TileContext(nc) as tc:
        with (
            tc.tile_pool(name="io", bufs=4) as io,
            tc.tile_pool(name="mid", bufs=3) as mid,
            tc.tile_pool(name="one", bufs=1) as one,
            tc.tile_pool(name="ps", bufs=1, space="PSUM") as ps,
        ):
            ones_b = one.tile([P, 1], bf16)
            nc.vector.memset(ones_b[:], 1.0)
            ones_f = one.tile([P, 1], f32)
            nc.vector.memset(ones_f[:], 1.0)
            ident = one.tile([P, P], bf16)
            nc.sync.dma_start(out=ident[:], in_=id_ext[:, :])
            a_st = one.tile([P, NT], f32)    # per-tile sum relu(t)^2 rows
            c_st = one.tile([P, NT], f32)    # per-tile mask counts

            psB = ps.tile([P, P], f32)       # accum r^T p   (diag wanted)
            psC = ps.tile([P, P], f32)       # accum m^T p^2 (diag wanted)
            g_tf = ps.tile([1, 2], f32)      # sum t at W-first cols {0, W}
            g_tl = ps.tile([1, 2], f32)      # sum t at W-last cols {W-1, TW-1}
            g_pf = ps.tile([1, 2], f32)
            g_pl = ps.tile([1, 2], f32)

            for i in range(NT):
                tb = io.tile([P, TW], bf16, tag="tb")
                pb = io.tile([P, TW], bf16, tag="pb")
                # SWDGE cast-DMA: f32 HBM -> bf16 SBUF
                nc.gpsimd.dma_start(out=tb[:], in_=t_ext[P * i:P * (i + 1), :])
                nc.gpsimd.dma_start(out=pb[:], in_=p_ext[P * i:P * (i + 1), :])

                r = mid.tile([P, TW], bf16, tag="r")
                sj = mid.tile([P, TW], bf16, tag="sj")
                m = mid.tile([P, TW], bf16, tag="m")
                p2 = mid.tile([P, TW], bf16, tag="p2")

                nc.scalar.activation(r[:], tb[:], Act.Relu)
                nc.scalar.activation(sj[:], r[:], Act.Square,
                                     accum_out=a_st[:, i:i + 1])
                nc.vector.tensor_scalar(m[:], tb[:], 0.0, None, Alu.is_gt,
                                        op1=Alu.add,
                                        accum_out=c_st[:, i:i + 1])
                nc.vector.tensor_tensor(p2[:], pb[:], pb[:], Alu.mult)

                for k in range(NCH):
                    sl = slice(P * k, P * (k + 1))
                    st = (i == 0 and k == 0)
                    sp = (i == NT - 1 and k == NCH - 1)
                    nc.tensor.matmul(psB[:], r[:, sl], pb[:, sl],
                                     start=st, stop=sp)
                    nc.tensor.matmul(psC[:], m[:, sl], p2[:, sl],
                                     start=st, stop=sp)

                st = (i == 0)
                sp = (i == NT - 1)
                nc.tensor.matmul(g_tf[:], ones_b[:], tb[:, 0:TW:W],
                                 start=st, stop=sp)
                nc.tensor.matmul(g_tl[:], ones_b[:], tb[:, W - 1:TW:W],
                                 start=st, stop=sp)
                nc.tensor.matmul(g_pf[:], ones_b[:], pb[:, 0:TW:W],
                                 start=st, stop=sp)
                nc.tensor.matmul(g_pl[:], ones_b[:], pb[:, W - 1:TW:W],
                                 start=st, stop=sp)

            # ---- tail: fold everything to one 16-float vector ----
            dgB = one.tile([P, P], f32)
            nc.vector.tensor_tensor(dgB[:], psB[:], ident[:], Alu.mult)
            dgC = one.tile([P, P], f32)
            nc.vector.tensor_tensor(dgC[:], psC[:], ident[:], Alu.mult)
            db = one.tile([P, 2], f32)
            nc.vector.tensor_reduce(db[:, 0:1], dgB[:], AxX, Alu.add)
            nc.vector.tensor_reduce(db[:, 1:2], dgC[:], AxX, Alu.add)
            apps = one.tile([P, 2], f32)
            nc.vector.tensor_reduce(apps[:, 0:1], a_st[:], AxX, Alu.add)
            nc.vector.tensor_reduce(apps[:, 1:2], c_st[:], AxX, Alu.add)

            # fin[:,0] = A - 2*diagB + diagC (per partition), fin[:,1] = count
            tmp = one.tile([P, 1], f32)
            nc.vector.scalar_tensor_tensor(tmp[:], db[:, 0:1], -2.0,
                                           db[:, 1:2], Alu.mult, Alu.add)
            fin = one.tile([P, 2], f32)
            nc.vector.tensor_tensor(fin[:, 0:1], tmp[:], apps[:, 0:1], Alu.add)
            nc.vector.tensor_copy(fin[:, 1:2], apps[:, 1:2])

            ps_fin = ps.tile([1, 2], f32)
            nc.tensor.matmul(ps_fin[:], ones_f[:], fin[:], start=True,
                             stop=True)

            outsb = one.tile([1, 16], f32)
            nc.vector.memset(outsb[:], 0.0)
            nc.vector.tensor_copy(outsb[0:1, 0:2], ps_fin[:])
            nc.vector.tensor_copy(outsb[0:1, 2:4], g_tf[:])
            nc.vector.tensor_copy(outsb[0:1, 4:6], g_tl[:])
            nc.vector.tensor_copy(outsb[0:1, 6:8], g_pf[:])
            nc.vector.tensor_copy(outsb[0:1, 8:10], g_pl[:])
            nc.sync.dma_start(out=out_ext[:, :], in_=outsb[:])

    nc.compile()
    return nc


def get_nc():
    if "nc" not in _CACHE:
        _CACHE["nc"] = _build_nc()
    return _CACHE["nc"]


def make_in_maps(pred, target):
    import ml_dtypes

    pred = np.ascontiguousarray(np.asarray(pred, dtype=np.float32))
    target = np.ascontiguousarray(np.asarray(target, dtype=np.float32))
    ident_np = np.eye(P, dtype=ml_dtypes.bfloat16)
    in_maps = []
    for c in range(NCORES):
        in_maps.append({
            "pred": pred[c * BP:(c + 1) * BP].reshape(RT, TW),
            "target": target[c * BP:(c + 1) * BP].reshape(RT, TW),
            "ident": ident_np,
        })
    return in_maps


def combine(results):
    """results: list (per core) of {"out": (1,16) f32} -> scalar loss."""
    S = NV = TF = TL = PF = PL = 0.0
    for c in range(NCORES):
        v = np.asarray(results[c]["out"], dtype=np.float64).reshape(16)
        S += v[0]
        NV += v[1]
        TF += v[2] + v[3]
        TL += v[4] + v[5]
        PF += v[6] + v[7]
        PL += v[8] + v[9]
    G = (TL - PL) - (TF - PF)
    loss = (1.0 - ALPHA) * (S / NV) + ALPHA * 128.0 * G / NTOT
    return np.asarray(loss, dtype=np.float32)


def kernel(pred, target):
    from concourse.bass_utils import run_bass_kernel_spmd

    nc = get_nc()
    in_maps = make_in_maps(pred, target)
    res = run_bass_kernel_spmd(nc, in_maps, core_ids=list(range(NCORES)))
    return combine(res.results)
